# revision 37
# baseline (speedup 1.0000x reference)
"""Trainium2 Bass kernel for nn_ActionRecognitionModel (relu-attention action model).

Math: the model's attention operates on a single-channel feature map Z >= 0
([B,1,T,V]); theta/void/g are outer products of Z's flattening with per-model
weight vectors, so the (VT x VT) relu-attention collapses exactly:

  Z[t,v]   = relu(vw.vel + vb) + relu(jw.joint + jb)          (>= 0)
  zvt      = Z flattened in (v,t) order, length VT = 8576
  s[a]     = sum_f w_theta[f] * zvt[134 f + a]      a in [0,134)
  u[j]     = w_void[j % 64] * s[j // 64]
  scores   = relu(theta @ void) = zvt_i * relu(u_j)           (Z >= 0)
  att[i,f] = w_g[f] * zvt_i * Sp,   Sp = sum_j relu(u_j) zvt_j
  logits   = q * (Sp * sumZ) + r * sumZ + t                   (q,r,t folded params)
  out      = softmax(logits)

With P[al] = sum_m relu(w_void)[m] * zvt[64 al + m] and N[al] likewise for
relu(-w_void), Sp = sum_al relu(s_al) P_al + relu(-s_al) N_al.

Both s and (P, N) are matmul contractions over *different* blockings of the
flat vector (134-blocks vs 64-blocks), which do not coexist in any single
2-D SBUF layout (67 and 128 are coprime).  Instead of materializing Z once
and reshaping through DRAM (two dependent DMA latencies), the host supplies
each input twice, pre-permuted (pure gather, no arithmetic), so the device
computes Z elementwise directly in the two matmul-friendly layouts:

  T2[p, w] = zvt[134*(p%64) + 67*(p//64) + w]   p in [0,128), w in [0,67)
     -> [s0, s1, -s0, -s1] via a wth stationary  (one matmul pair)
  Z3[q, w] = zvt[4288*(q//64) + 64*w + (q%64)]
     -> [P0, P1, N0, N1] via a w_void stationary (one matmul pair)

The madd chains for all four permuted inputs run on the Pool engine (whose
queue-ordered DMAs let compute start right after the DMA issue slices); the
two T2 relus run on DVE to use its free-dim accumulator for the row sums
that the host reduces to sumZ.  One fused DVE op then computes the Sp
partials max(+-s,0)*[P|N] against pn_ps in PSUM, and a single DMA per
output ships [67] partials; the host folds the final Linear+softmax over
the 100 classes (all class weights are host-folded scalars q, r, t).

Each core computes one batch end-to-end on device (data parallel over B,
replicated 4x across the 8 cores).
"""

import numpy as np

try:
    import concourse.bass as bass
except ImportError:  # fallback if the axon site hook isn't installed
    import sys

    sys.path.insert(0, "/opt/trn_rl_repo")
    import concourse.bass as bass

import concourse.bacc as bacc
import concourse.tile as tile
from concourse import mybir
from concourse.bass_utils import run_bass_kernel_spmd

F32 = mybir.dt.float32
BF16 = mybir.dt.bfloat16
ALU = mybir.AluOpType

B, C, T, V, F, NCLS = 2, 4, 128, 67, 64, 100
VT = V * T  # 8576
A = VT // F  # 134

# csts layout ([128, 8] bf16): cols 0:4 = s-matmul stationary producing
# [s0, s1, -s0, -s1] (col h has +/-w_theta[f] at row 64h+f); cols 4:8 = PN
# stationary producing [P0, P1, N0, N1] (relu(w_void) / relu(-w_void)).
N_CSTS = 8

# host-side gather indices for the two device layouts (pure permutations)
_p = np.arange(128)[:, None]
_w = np.arange(V)[None, :]
_jT2 = 134 * (_p % 64) + 67 * (_p // 64) + _w  # [128, 67]
_jZ3 = 4288 * (_p // 64) + 64 * _w + (_p % 64)  # [128, 67]
_T2_T, _T2_V = _jT2 % T, _jT2 // T
_Z3_T, _Z3_V = _jZ3 % T, _jZ3 // T

_NC_CACHE = {}


def build_nc(vw, vb, jw, jb):
    vw = [float(x) for x in vw]
    jw = [float(x) for x in jw]
    vb, jb = float(vb), float(jb)
    nc = bacc.Bacc(None, target_bir_lowering=False)
    velT2 = nc.dram_tensor("velT2", [T, C, V], BF16, kind="ExternalInput")
    jntT2 = nc.dram_tensor("jntT2", [T, C, V], BF16, kind="ExternalInput")
    velZ3 = nc.dram_tensor("velZ3", [T, C, V], BF16, kind="ExternalInput")
    jntZ3 = nc.dram_tensor("jntZ3", [T, C, V], BF16, kind="ExternalInput")
    csts = nc.dram_tensor("csts", [T, N_CSTS], BF16, kind="ExternalInput")
    outa = nc.dram_tensor("outa", [V, 1], F32, kind="ExternalOutput")
    outb = nc.dram_tensor("outb", [T, 2], BF16, kind="ExternalOutput")

    with tile.TileContext(nc) as tc:
        with (
            tc.tile_pool(name="work", bufs=1) as work,
            tc.tile_pool(name="psum", bufs=1, space="PSUM") as psum,
        ):
            # --- input DMAs: only the tiny constants ride the Pool queue, so
            # its chains start right after that slice; the four permuted
            # inputs split across the two HWDGE queues in chain-use order ---
            cs = work.tile([T, N_CSTS], BF16, name="cs")
            nc.gpsimd.dma_start(out=cs[:], in_=csts[:])
            velT2_sb = work.tile([T, C, V], BF16, name="velT2_sb")
            nc.sync.dma_start(out=velT2_sb[:], in_=velT2[:])
            jntT2_sb = work.tile([T, C, V], BF16, name="jntT2_sb")
            nc.scalar.dma_start(out=jntT2_sb[:], in_=jntT2[:])
            velZ3_sb = work.tile([T, C, V], BF16, name="velZ3_sb")
            nc.sync.dma_start(out=velZ3_sb[:], in_=velZ3[:])
            jntZ3_sb = work.tile([T, C, V], BF16, name="jntZ3_sb")
            nc.scalar.dma_start(out=jntZ3_sb[:], in_=jntZ3[:])

            # --- all four madd chains on Pool (no fused scalar_tensor_tensor
            # opcode there, so each madd is a mul + add pair; conv weights
            # baked as immediates). T2 chains stop before the relu, which
            # runs on DVE to get the free-dim accumulator for sumZ. ---
            def pool_chain(src, w, b, z_name, relu):
                z = work.tile([T, V], BF16, name=z_name)
                nc.gpsimd.tensor_scalar(z[:], src[:, 0, :], w[0], b,
                                        op0=ALU.mult, op1=ALU.add)
                tmp = work.tile([T, V], BF16, name=z_name + "_t")
                for c in range(1, C):
                    nc.gpsimd.tensor_scalar_mul(tmp[:], src[:, c, :], w[c])
                    nc.gpsimd.tensor_add(z[:], z[:], tmp[:])
                if not relu:
                    return z
                r = work.tile([T, V], BF16, name=z_name + "r")
                nc.gpsimd.tensor_scalar_max(r[:], z[:], 0.0)
                return r

            t2v = pool_chain(velT2_sb, vw, vb, "t2v", relu=False)
            t2j = pool_chain(jntT2_sb, jw, jb, "t2j", relu=False)
            Z3v = pool_chain(velZ3_sb, vw, vb, "z3v", relu=True)
            Z3j = pool_chain(jntZ3_sb, jw, jb, "z3j", relu=True)

            rs_sb = work.tile([T, 2], BF16, name="rs_sb")
            T2v = work.tile([T, V], BF16, name="T2v")
            nc.vector.tensor_scalar(T2v[:], t2v[:], 0.0, 0.0, op0=ALU.max,
                                    op1=ALU.add, accum_out=rs_sb[:, 0:1])
            T2j = work.tile([T, V], BF16, name="T2j")
            nc.vector.tensor_scalar(T2j[:], t2j[:], 0.0, 0.0, op0=ALU.max,
                                    op1=ALU.add, accum_out=rs_sb[:, 1:2])

            # --- PE: s and PN contractions, each accumulating the vel/joint
            # halves in PSUM (Z = Zv + Zj never materialized) ---
            s_ps = psum.tile([V, 4], F32, name="s_ps")
            nc.tensor.matmul(s_ps[:], T2v[:], cs[:, 0:4], start=True, stop=False)
            nc.tensor.matmul(s_ps[:], T2j[:], cs[:, 0:4], start=False, stop=True)
            pn_ps = psum.tile([V, 4], F32, name="pn_ps")
            nc.tensor.matmul(pn_ps[:], Z3v[:], cs[:, 4:8], start=True, stop=False)
            nc.tensor.matmul(pn_ps[:], Z3j[:], cs[:, 4:8], start=False, stop=True)

            # --- combine: Sp partials = sum_cols max(+-s,0) * [P|N]. s is
            # staged to SBUF (it is ready early), so the combine's single
            # PSUM operand can be pn_ps (ready late). ---
            s_sb = work.tile([V, 4], F32, name="s_sb")
            nc.vector.tensor_copy(s_sb[:], s_ps[:])
            accs = work.tile([V, 1], F32, name="accs")
            junk = work.tile([V, 4], F32, name="junk")
            nc.vector.scalar_tensor_tensor(
                junk[:], s_sb[:], 0.0, pn_ps[:], op0=ALU.max, op1=ALU.mult,
                accum_out=accs[:])

            # --- outputs: row sums on the scalar queue, combine partials on
            # the sync queue ---
            nc.scalar.dma_start(out=outb[:], in_=rs_sb[:])
            nc.sync.dma_start(out=outa[:], in_=accs[:])
    nc.compile()
    return nc


def get_nc(vw, vb, jw, jb):
    key = (tuple(np.float32(x) for x in vw), np.float32(vb),
           tuple(np.float32(x) for x in jw), np.float32(jb))
    if key not in _NC_CACHE:
        _NC_CACHE[key] = build_nc(vw, vb, jw, jb)
    return _NC_CACHE[key]


def _fold(vc1_w, vc1_b, vc2_w, vc2_b, sc1_w, sc1_b, sc2_w, sc2_b,
          w_theta, w_void, w_g, convh_w, convh_b, lin_w, lin_b):
    f32 = np.float32
    vw = (vc2_w[0, 0] * vc1_w[0]).astype(f32)
    vb = f32(vc2_w[0, 0] * vc1_b[0] + vc2_b[0])
    jw = (sc2_w[0, 0] * sc1_w[0]).astype(f32)
    jb = f32(sc2_w[0, 0] * sc1_b[0] + sc2_b[0])

    wvp = np.maximum(w_void, 0).astype(f32)
    wvn = np.maximum(-w_void, 0).astype(f32)
    csts = np.zeros((T, N_CSTS), f32)
    csts[:F, 0] = w_theta
    csts[F:, 1] = w_theta
    csts[:F, 2] = -w_theta
    csts[F:, 3] = -w_theta
    csts[:F, 4] = wvp
    csts[F:, 5] = wvp
    csts[:F, 6] = wvn
    csts[F:, 7] = wvn

    cw = convh_w @ w_g
    q = (lin_w @ cw) / VT
    r = lin_w.sum(axis=1) / VT
    t = lin_w @ convh_b + lin_b
    return vw, vb, jw, jb, csts, q, r, t


def kernel(**inputs):
    f32 = np.float32
    joint_matrix = inputs.pop("joint_matrix")
    vel_matrix = inputs.pop("vel_matrix")
    vw, vb, jw, jb, csts, q, r, t = _fold(**inputs)
    nc = get_nc(vw, vb, jw, jb)

    import ml_dtypes

    bf16 = ml_dtypes.bfloat16
    per_batch = []
    for b in range(B):
        vel, joint = vel_matrix[b], joint_matrix[b]
        per_batch.append({
            "velZ3": np.ascontiguousarray(vel[:, _Z3_T, _Z3_V].transpose(1, 0, 2), bf16),
            "jntZ3": np.ascontiguousarray(joint[:, _Z3_T, _Z3_V].transpose(1, 0, 2), bf16),
            "velT2": np.ascontiguousarray(vel[:, _T2_T, _T2_V].transpose(1, 0, 2), bf16),
            "jntT2": np.ascontiguousarray(joint[:, _T2_T, _T2_V].transpose(1, 0, 2), bf16),
            "csts": csts.astype(bf16),
        })
    in_maps = [per_batch[k % B] for k in range(8)]

    last_exc = None
    for attempt in range(3):
        try:
            res = run_bass_kernel_spmd(nc, in_maps, core_ids=list(range(8)))
            break
        except Exception as exc:  # transient NRT/device hiccups recover on retry
            last_exc = exc
            if attempt == 2:
                raise
            import time

            time.sleep(10)

    out = np.zeros((B, NCLS), f32)
    for b in range(B):
        outa = res.results[b]["outa"]  # [67, 1] combine partials
        outb = res.results[b]["outb"]  # [128, 2] row sums (bf16)
        Sp = f32(outa.astype(f32).sum())
        sumZ = f32(outb.astype(f32).sum())
        logits = q * (Sp * sumZ) + r * sumZ + t
        e = np.exp(logits - logits.max())
        out[b] = e / e.sum()
    return out.astype(f32)


# revision 38
# speedup vs baseline: 1.0702x; 1.0702x over previous
"""Trainium2 Bass kernel for nn_ActionRecognitionModel (relu-attention action model).

Math: the model's attention operates on a single-channel feature map Z >= 0
([B,1,T,V]); theta/void/g are outer products of Z's flattening with per-model
weight vectors, so the (VT x VT) relu-attention collapses exactly:

  Z[t,v]   = relu(vw.vel + vb) + relu(jw.joint + jb)          (>= 0)
  zvt      = Z flattened in (v,t) order, length VT = 8576
  s[a]     = sum_f w_theta[f] * zvt[134 f + a]      a in [0,134)
  u[j]     = w_void[j % 64] * s[j // 64]
  scores   = relu(theta @ void) = zvt_i * relu(u_j)           (Z >= 0)
  att[i,f] = w_g[f] * zvt_i * Sp,   Sp = sum_j relu(u_j) zvt_j
  logits   = q * (Sp * sumZ) + r * sumZ + t                   (q,r,t folded params)
  out      = softmax(logits)

With P[al] = sum_m relu(w_void)[m] * zvt[64 al + m] and N[al] likewise for
relu(-w_void), Sp = sum_al relu(s_al) P_al + relu(-s_al) N_al.

Both s and (P, N) are matmul contractions over *different* blockings of the
flat vector (134-blocks vs 64-blocks), which do not coexist in any single
2-D SBUF layout (67 and 128 are coprime).  Instead of materializing Z once
and reshaping through DRAM (two dependent DMA latencies), the host supplies
each input twice, pre-permuted (pure gather, no arithmetic), so the device
computes Z elementwise directly in the two matmul-friendly layouts:

  T2[p, w] = zvt[134*(p%64) + 67*(p//64) + w]   p in [0,128), w in [0,67)
     -> [s0, s1, -s0, -s1] via a wth stationary  (one matmul pair)
  Z3[q, w] = zvt[4288*(q//64) + 64*w + (q%64)]
     -> [P0, P1, N0, N1] via a w_void stationary (one matmul pair)

The madd chains for all four permuted inputs run on the Pool engine (whose
queue-ordered DMAs let compute start right after the DMA issue slices); the
two T2 relus run on DVE to use its free-dim accumulator for the row sums
that the host reduces to sumZ.  One fused DVE op then computes the Sp
partials max(+-s,0)*[P|N] against pn_ps in PSUM, and a single DMA per
output ships [67] partials; the host folds the final Linear+softmax over
the 100 classes (all class weights are host-folded scalars q, r, t).

Each core computes one batch end-to-end on device (data parallel over B,
replicated 4x across the 8 cores).
"""

import numpy as np

try:
    import concourse.bass as bass
except ImportError:  # fallback if the axon site hook isn't installed
    import sys

    sys.path.insert(0, "/opt/trn_rl_repo")
    import concourse.bass as bass

import concourse.bacc as bacc
import concourse.tile as tile
from concourse import mybir
from concourse.bass_utils import run_bass_kernel_spmd

F32 = mybir.dt.float32
BF16 = mybir.dt.bfloat16
ALU = mybir.AluOpType

B, C, T, V, F, NCLS = 2, 4, 128, 67, 64, 100
VT = V * T  # 8576
A = VT // F  # 134

# csts layout ([128, 8] bf16): cols 0:4 = s-matmul stationary producing
# [s0, s1, -s0, -s1] (col h has +/-w_theta[f] at row 64h+f); cols 4:8 = PN
# stationary producing [P0, P1, N0, N1] (relu(w_void) / relu(-w_void)).
N_CSTS = 8

# host-side gather indices for the two device layouts (pure permutations)
_p = np.arange(128)[:, None]
_w = np.arange(V)[None, :]
_jT2 = 134 * (_p % 64) + 67 * (_p // 64) + _w  # [128, 67]
_jZ3 = 4288 * (_p // 64) + 64 * _w + (_p % 64)  # [128, 67]
_T2_T, _T2_V = _jT2 % T, _jT2 // T
_Z3_T, _Z3_V = _jZ3 % T, _jZ3 // T

_NC_CACHE = {}


def build_nc(vw, vb, jw, jb):
    vw = [float(x) for x in vw]
    jw = [float(x) for x in jw]
    vb, jb = float(vb), float(jb)
    nc = bacc.Bacc(None, target_bir_lowering=False)
    velT2 = nc.dram_tensor("velT2", [T, C, V], BF16, kind="ExternalInput")
    jntT2 = nc.dram_tensor("jntT2", [T, C, V], BF16, kind="ExternalInput")
    velZ3 = nc.dram_tensor("velZ3", [T, C, V], BF16, kind="ExternalInput")
    jntZ3 = nc.dram_tensor("jntZ3", [T, C, V], BF16, kind="ExternalInput")
    csts = nc.dram_tensor("csts", [T, N_CSTS], BF16, kind="ExternalInput")
    outa = nc.dram_tensor("outa", [V, 1], F32, kind="ExternalOutput")
    outb = nc.dram_tensor("outb", [T, 2], BF16, kind="ExternalOutput")

    with tile.TileContext(nc) as tc:
        with (
            tc.tile_pool(name="work", bufs=1) as work,
            tc.tile_pool(name="psum", bufs=1, space="PSUM") as psum,
        ):
            # --- input DMAs. Scheduling rule this layout exploits: a
            # consumer op only pays a DMA-completion wait if it would
            # dispatch before the DMA's issue slice ends, so every engine's
            # first input-consuming op is arranged to dispatch after the
            # slices of the inputs it reads (Pool via its own queued DMA,
            # DVE via a dummy delay op). ---
            velZ3_sb = work.tile([T, C, V], BF16, name="velZ3_sb")
            nc.gpsimd.dma_start(out=velZ3_sb[:], in_=velZ3[:])
            velT2_sb = work.tile([T, C, V], BF16, name="velT2_sb")
            nc.sync.dma_start(out=velT2_sb[:], in_=velT2[:])
            cs = work.tile([T, N_CSTS], BF16, name="cs")
            nc.sync.dma_start(out=cs[:], in_=csts[:])
            jntT2_sb = work.tile([T, C, V], BF16, name="jntT2_sb")
            nc.scalar.dma_start(out=jntT2_sb[:], in_=jntT2[:])
            jntZ3_sb = work.tile([T, C, V], BF16, name="jntZ3_sb")
            nc.scalar.dma_start(out=jntZ3_sb[:], in_=jntZ3[:])

            # --- madd chains, split Pool / DVE so they run concurrently.
            # Pool lacks the fused scalar_tensor_tensor opcode, so its madds
            # are mul + add pairs; conv weights are baked as immediates. T2
            # relus run on DVE for the free-dim accumulator (sumZ row sums).
            def pool_chain(src, w, b, z_name, relu):
                z = work.tile([T, V], BF16, name=z_name)
                nc.gpsimd.tensor_scalar(z[:], src[:, 0, :], w[0], b,
                                        op0=ALU.mult, op1=ALU.add)
                tmp = work.tile([T, V], BF16, name=z_name + "_t")
                for c in range(1, C):
                    nc.gpsimd.tensor_scalar_mul(tmp[:], src[:, c, :], w[c])
                    nc.gpsimd.tensor_add(z[:], z[:], tmp[:])
                if not relu:
                    return z
                r = work.tile([T, V], BF16, name=z_name + "r")
                nc.gpsimd.tensor_scalar_max(r[:], z[:], 0.0)
                return r

            # DVE is delayed past the velT2 slice by a dummy memset, then
            # runs the velT2 madds while Pool works through the other three.
            dummy = work.tile([T, 600], F32, name="dummy")
            nc.vector.memset(dummy[:], 0.0)
            t2v = work.tile([T, V], BF16, name="t2v")
            nc.vector.tensor_scalar(t2v[:], velT2_sb[:, 0, :], vw[0], vb,
                                    op0=ALU.mult, op1=ALU.add)
            for c in range(1, C):
                nc.vector.scalar_tensor_tensor(
                    t2v[:], velT2_sb[:, c, :], vw[c], t2v[:],
                    op0=ALU.mult, op1=ALU.add)

            Z3v = pool_chain(velZ3_sb, vw, vb, "z3v", relu=True)
            t2j = pool_chain(jntT2_sb, jw, jb, "t2j", relu=False)
            Z3j = pool_chain(jntZ3_sb, jw, jb, "z3j", relu=True)

            rs_sb = work.tile([T, 2], BF16, name="rs_sb")
            T2v = work.tile([T, V], BF16, name="T2v")
            nc.vector.tensor_scalar(T2v[:], t2v[:], 0.0, 0.0, op0=ALU.max,
                                    op1=ALU.add, accum_out=rs_sb[:, 0:1])
            T2j = work.tile([T, V], BF16, name="T2j")
            nc.vector.tensor_scalar(T2j[:], t2j[:], 0.0, 0.0, op0=ALU.max,
                                    op1=ALU.add, accum_out=rs_sb[:, 1:2])

            # --- PE: s and PN contractions, each accumulating the vel/joint
            # halves in PSUM (Z = Zv + Zj never materialized) ---
            s_ps = psum.tile([V, 4], F32, name="s_ps")
            nc.tensor.matmul(s_ps[:], T2v[:], cs[:, 0:4], start=True, stop=False)
            pn_ps = psum.tile([V, 4], F32, name="pn_ps")
            nc.tensor.matmul(pn_ps[:], Z3v[:], cs[:, 4:8], start=True, stop=False)
            nc.tensor.matmul(s_ps[:], T2j[:], cs[:, 0:4], start=False, stop=True)
            nc.tensor.matmul(pn_ps[:], Z3j[:], cs[:, 4:8], start=False, stop=True)

            # --- combine: Sp partials = sum_cols max(+-s,0) * [P|N]. s is
            # staged to SBUF (it is ready early), so the combine's single
            # PSUM operand can be pn_ps (ready late). ---
            s_sb = work.tile([V, 4], F32, name="s_sb")
            nc.vector.tensor_copy(s_sb[:], s_ps[:])
            accs = work.tile([V, 1], F32, name="accs")
            junk = work.tile([V, 4], F32, name="junk")
            nc.vector.scalar_tensor_tensor(
                junk[:], s_sb[:], 0.0, pn_ps[:], op0=ALU.max, op1=ALU.mult,
                accum_out=accs[:])

            # --- outputs: row sums on the scalar queue, combine partials on
            # the sync queue ---
            nc.scalar.dma_start(out=outb[:], in_=rs_sb[:])
            nc.sync.dma_start(out=outa[:], in_=accs[:])
    nc.compile()
    return nc


def get_nc(vw, vb, jw, jb):
    key = (tuple(np.float32(x) for x in vw), np.float32(vb),
           tuple(np.float32(x) for x in jw), np.float32(jb))
    if key not in _NC_CACHE:
        _NC_CACHE[key] = build_nc(vw, vb, jw, jb)
    return _NC_CACHE[key]


def _fold(vc1_w, vc1_b, vc2_w, vc2_b, sc1_w, sc1_b, sc2_w, sc2_b,
          w_theta, w_void, w_g, convh_w, convh_b, lin_w, lin_b):
    f32 = np.float32
    vw = (vc2_w[0, 0] * vc1_w[0]).astype(f32)
    vb = f32(vc2_w[0, 0] * vc1_b[0] + vc2_b[0])
    jw = (sc2_w[0, 0] * sc1_w[0]).astype(f32)
    jb = f32(sc2_w[0, 0] * sc1_b[0] + sc2_b[0])

    wvp = np.maximum(w_void, 0).astype(f32)
    wvn = np.maximum(-w_void, 0).astype(f32)
    csts = np.zeros((T, N_CSTS), f32)
    csts[:F, 0] = w_theta
    csts[F:, 1] = w_theta
    csts[:F, 2] = -w_theta
    csts[F:, 3] = -w_theta
    csts[:F, 4] = wvp
    csts[F:, 5] = wvp
    csts[:F, 6] = wvn
    csts[F:, 7] = wvn

    cw = convh_w @ w_g
    q = (lin_w @ cw) / VT
    r = lin_w.sum(axis=1) / VT
    t = lin_w @ convh_b + lin_b
    return vw, vb, jw, jb, csts, q, r, t


def kernel(**inputs):
    f32 = np.float32
    joint_matrix = inputs.pop("joint_matrix")
    vel_matrix = inputs.pop("vel_matrix")
    vw, vb, jw, jb, csts, q, r, t = _fold(**inputs)
    nc = get_nc(vw, vb, jw, jb)

    import ml_dtypes

    bf16 = ml_dtypes.bfloat16
    per_batch = []
    for b in range(B):
        vel, joint = vel_matrix[b], joint_matrix[b]
        per_batch.append({
            "velZ3": np.ascontiguousarray(vel[:, _Z3_T, _Z3_V].transpose(1, 0, 2), bf16),
            "jntZ3": np.ascontiguousarray(joint[:, _Z3_T, _Z3_V].transpose(1, 0, 2), bf16),
            "velT2": np.ascontiguousarray(vel[:, _T2_T, _T2_V].transpose(1, 0, 2), bf16),
            "jntT2": np.ascontiguousarray(joint[:, _T2_T, _T2_V].transpose(1, 0, 2), bf16),
            "csts": csts.astype(bf16),
        })
    in_maps = [per_batch[k % B] for k in range(8)]

    last_exc = None
    for attempt in range(3):
        try:
            res = run_bass_kernel_spmd(nc, in_maps, core_ids=list(range(8)))
            break
        except Exception as exc:  # transient NRT/device hiccups recover on retry
            last_exc = exc
            if attempt == 2:
                raise
            import time

            time.sleep(10)

    out = np.zeros((B, NCLS), f32)
    for b in range(B):
        outa = res.results[b]["outa"]  # [67, 1] combine partials
        outb = res.results[b]["outb"]  # [128, 2] row sums (bf16)
        Sp = f32(outa.astype(f32).sum())
        sumZ = f32(outb.astype(f32).sum())
        logits = q * (Sp * sumZ) + r * sumZ + t
        e = np.exp(logits - logits.max())
        out[b] = e / e.sum()
    return out.astype(f32)


# revision 40
# speedup vs baseline: 1.4038x; 1.3118x over previous
"""Trainium2 Bass kernel for nn_ActionRecognitionModel (relu-attention action model).

Math: the model's attention operates on a single-channel feature map Z >= 0
([B,1,T,V]); theta/void/g are outer products of Z's flattening with per-model
weight vectors, so the (VT x VT) relu-attention collapses exactly:

  Z[t,v]   = relu(vw.vel + vb) + relu(jw.joint + jb)          (>= 0)
  zvt      = Z flattened in (v,t) order, length VT = 8576
  s[a]     = sum_f w_theta[f] * zvt[134 f + a]      a in [0,134)
  u[j]     = w_void[j % 64] * s[j // 64]
  scores   = relu(theta @ void) = zvt_i * relu(u_j)           (Z >= 0)
  att[i,f] = w_g[f] * zvt_i * Sp,   Sp = sum_j relu(u_j) zvt_j
  logits   = q * (Sp * sumZ) + r * sumZ + t                   (q,r,t folded params)
  out      = softmax(logits)

With P[al] = sum_m relu(w_void)[m] * zvt[64 al + m] and N[al] likewise for
relu(-w_void), Sp = sum_al relu(s_al) P_al + relu(-s_al) N_al.

Both s and (P, N) are matmul contractions over *different* blockings of the
flat vector (134-blocks vs 64-blocks), which do not coexist in any single
2-D SBUF layout (67 and 128 are coprime).  Instead of materializing Z once
and reshaping through DRAM (two dependent DMA latencies), the host supplies
each input twice, pre-permuted (pure gather, no arithmetic), so the device
computes Z elementwise directly in the two matmul-friendly layouts:

  T2[p, w] = zvt[134*(p%64) + 67*(p//64) + w]   p in [0,128), w in [0,67)
     -> [s0, s1, -s0, -s1] via a wth stationary  (one matmul pair)
  Z3[q, w] = zvt[4288*(q//64) + 64*w + (q%64)]
     -> [P0, P1, N0, N1] via a w_void stationary (one matmul pair)

The madd chains for all four permuted inputs run on the Pool engine (whose
queue-ordered DMAs let compute start right after the DMA issue slices); the
two T2 relus run on DVE to use its free-dim accumulator for the row sums
that the host reduces to sumZ.  One fused DVE op then computes the Sp
partials max(+-s,0)*[P|N] against pn_ps in PSUM, and a single DMA per
output ships [67] partials; the host folds the final Linear+softmax over
the 100 classes (all class weights are host-folded scalars q, r, t).

Each core computes one batch end-to-end on device (data parallel over B,
replicated 4x across the 8 cores).
"""

import numpy as np

try:
    import concourse.bass as bass
except ImportError:  # fallback if the axon site hook isn't installed
    import sys

    sys.path.insert(0, "/opt/trn_rl_repo")
    import concourse.bass as bass

import concourse.bacc as bacc
import concourse.tile as tile
from concourse import mybir
from concourse.bass_utils import run_bass_kernel_spmd

F32 = mybir.dt.float32
BF16 = mybir.dt.bfloat16
ALU = mybir.AluOpType

B, C, T, V, F, NCLS = 2, 4, 128, 67, 64, 100
VT = V * T  # 8576
A = VT // F  # 134

# csts layout ([128, 8] bf16): cols 0:4 = s-matmul stationary producing
# [s0, s1, -s0, -s1] (col h has +/-w_theta[f] at row 64h+f); cols 4:8 = PN
# stationary producing [P0, P1, N0, N1] (relu(w_void) / relu(-w_void)).
N_CSTS = 8

# host-side gather indices for the two device layouts (pure permutations)
_p = np.arange(128)[:, None]
_w = np.arange(V)[None, :]
_jT2 = 134 * (_p % 64) + 67 * (_p // 64) + _w  # [128, 67]
_jZ3 = 4288 * (_p // 64) + 64 * _w + (_p % 64)  # [128, 67]
_T2_T, _T2_V = _jT2 % T, _jT2 // T
_Z3_T, _Z3_V = _jZ3 % T, _jZ3 // T

_NC_CACHE = {}


def build_nc(vw, vb, jw, jb):
    vw = [float(x) for x in vw]
    jw = [float(x) for x in jw]
    vb, jb = float(vb), float(jb)
    nc = bacc.Bacc(None, target_bir_lowering=False)
    velT2 = nc.dram_tensor("velT2", [T, C, V], BF16, kind="ExternalInput")
    jntT2 = nc.dram_tensor("jntT2", [T, C, V], BF16, kind="ExternalInput")
    velZ3 = nc.dram_tensor("velZ3", [T, C, V], BF16, kind="ExternalInput")
    jntZ3 = nc.dram_tensor("jntZ3", [T, C, V], BF16, kind="ExternalInput")
    csts = nc.dram_tensor("csts", [T, N_CSTS], BF16, kind="ExternalInput")
    outa = nc.dram_tensor("outa", [V, 1], F32, kind="ExternalOutput")
    outb = nc.dram_tensor("outb", [T, 2], BF16, kind="ExternalOutput")

    with tile.TileContext(nc) as tc:
        with (
            tc.tile_pool(name="work", bufs=1) as work,
            tc.tile_pool(name="psum", bufs=1, space="PSUM") as psum,
        ):
            # --- input DMAs. Scheduling rule this layout exploits: a
            # consumer op only pays a DMA-completion wait if it would
            # dispatch before the DMA's issue slice ends, so every engine's
            # first input-consuming op is arranged to dispatch after the
            # slices of the inputs it reads (Pool via its own queued DMA,
            # DVE via a dummy delay op). ---
            velZ3_sb = work.tile([T, C, V], BF16, name="velZ3_sb")
            nc.gpsimd.dma_start(out=velZ3_sb[:], in_=velZ3[:])
            velT2_sb = work.tile([T, C, V], BF16, name="velT2_sb")
            nc.sync.dma_start(out=velT2_sb[:], in_=velT2[:])
            cs = work.tile([T, N_CSTS], BF16, name="cs")
            nc.sync.dma_start(out=cs[:], in_=csts[:])
            jntT2_sb = work.tile([T, C, V], BF16, name="jntT2_sb")
            nc.scalar.dma_start(out=jntT2_sb[:], in_=jntT2[:])
            jntZ3_sb = work.tile([T, C, V], BF16, name="jntZ3_sb")
            nc.scalar.dma_start(out=jntZ3_sb[:], in_=jntZ3[:])

            # --- madd chains, split Pool / DVE so they run concurrently.
            # Pool lacks the fused scalar_tensor_tensor opcode, so its madds
            # are mul + add pairs; conv weights are baked as immediates. T2
            # relus run on DVE for the free-dim accumulator (sumZ row sums).
            # One tmp tile shared by all Pool chains: each chain's first op
            # writes tmp, whose WAR hazard against the previous chain's last
            # read serializes the chains in program order. That keeps every
            # input-reading op dispatching after its DMA's issue slice, so
            # no completion waits are emitted.
            tmp = work.tile([T, V], BF16, name="pool_tmp")

            def pool_chain(src, w, b, z_name, relu, lead):
                z = work.tile([T, V], BF16, name=z_name)
                if lead:
                    # first op must be the tmp write (chain-serializing dep)
                    nc.gpsimd.tensor_scalar_mul(tmp[:], src[:, 0, :], w[0])
                    nc.gpsimd.tensor_scalar_add(z[:], tmp[:], b)
                else:
                    nc.gpsimd.tensor_scalar(z[:], src[:, 0, :], w[0], b,
                                            op0=ALU.mult, op1=ALU.add)
                for c in range(1, C):
                    nc.gpsimd.tensor_scalar_mul(tmp[:], src[:, c, :], w[c])
                    nc.gpsimd.tensor_add(z[:], z[:], tmp[:])
                if not relu:
                    return z
                r = work.tile([T, V], BF16, name=z_name + "r")
                nc.gpsimd.tensor_scalar_max(r[:], z[:], 0.0)
                return r

            # DVE is delayed past the velT2 slice by a dummy memset, then
            # runs the velT2 madds while Pool works through the other three.
            dummy = work.tile([T, 600], F32, name="dummy")
            nc.vector.memset(dummy[:], 0.0)
            t2v = work.tile([T, V], BF16, name="t2v")
            nc.vector.tensor_scalar(t2v[:], velT2_sb[:, 0, :], vw[0], vb,
                                    op0=ALU.mult, op1=ALU.add)
            for c in range(1, C):
                nc.vector.scalar_tensor_tensor(
                    t2v[:], velT2_sb[:, c, :], vw[c], t2v[:],
                    op0=ALU.mult, op1=ALU.add)

            Z3v = pool_chain(velZ3_sb, vw, vb, "z3v", relu=True, lead=False)
            t2j = pool_chain(jntT2_sb, jw, jb, "t2j", relu=False, lead=True)
            Z3j = pool_chain(jntZ3_sb, jw, jb, "z3j", relu=True, lead=True)

            rs_sb = work.tile([T, 2], BF16, name="rs_sb")
            T2v = work.tile([T, V], BF16, name="T2v")
            nc.vector.tensor_scalar(T2v[:], t2v[:], 0.0, 0.0, op0=ALU.max,
                                    op1=ALU.add, accum_out=rs_sb[:, 0:1])
            T2j = work.tile([T, V], BF16, name="T2j")
            nc.vector.tensor_scalar(T2j[:], t2j[:], 0.0, 0.0, op0=ALU.max,
                                    op1=ALU.add, accum_out=rs_sb[:, 1:2])

            # --- PE: s and PN contractions, each accumulating the vel/joint
            # halves in PSUM (Z = Zv + Zj never materialized) ---
            s_ps = psum.tile([V, 4], F32, name="s_ps")
            nc.tensor.matmul(s_ps[:], T2v[:], cs[:, 0:4], start=True, stop=False)
            pn_ps = psum.tile([V, 4], F32, name="pn_ps")
            nc.tensor.matmul(pn_ps[:], Z3v[:], cs[:, 4:8], start=True, stop=False)
            nc.tensor.matmul(s_ps[:], T2j[:], cs[:, 0:4], start=False, stop=True)
            nc.tensor.matmul(pn_ps[:], Z3j[:], cs[:, 4:8], start=False, stop=True)

            # --- combine: Sp partials = sum_cols max(+-s,0) * [P|N]. s is
            # staged to SBUF (it is ready early), so the combine's single
            # PSUM operand can be pn_ps (ready late). ---
            s_sb = work.tile([V, 4], F32, name="s_sb")
            nc.vector.tensor_copy(s_sb[:], s_ps[:])
            accs = work.tile([V, 1], F32, name="accs")
            junk = work.tile([V, 4], F32, name="junk")
            nc.vector.scalar_tensor_tensor(
                junk[:], s_sb[:], 0.0, pn_ps[:], op0=ALU.max, op1=ALU.mult,
                accum_out=accs[:])

            # --- outputs: row sums on the scalar queue, combine partials on
            # the sync queue ---
            nc.scalar.dma_start(out=outb[:], in_=rs_sb[:])
            nc.sync.dma_start(out=outa[:], in_=accs[:])
    nc.compile()
    return nc


def get_nc(vw, vb, jw, jb):
    key = (tuple(np.float32(x) for x in vw), np.float32(vb),
           tuple(np.float32(x) for x in jw), np.float32(jb))
    if key not in _NC_CACHE:
        _NC_CACHE[key] = build_nc(vw, vb, jw, jb)
    return _NC_CACHE[key]


def _fold(vc1_w, vc1_b, vc2_w, vc2_b, sc1_w, sc1_b, sc2_w, sc2_b,
          w_theta, w_void, w_g, convh_w, convh_b, lin_w, lin_b):
    f32 = np.float32
    vw = (vc2_w[0, 0] * vc1_w[0]).astype(f32)
    vb = f32(vc2_w[0, 0] * vc1_b[0] + vc2_b[0])
    jw = (sc2_w[0, 0] * sc1_w[0]).astype(f32)
    jb = f32(sc2_w[0, 0] * sc1_b[0] + sc2_b[0])

    wvp = np.maximum(w_void, 0).astype(f32)
    wvn = np.maximum(-w_void, 0).astype(f32)
    csts = np.zeros((T, N_CSTS), f32)
    csts[:F, 0] = w_theta
    csts[F:, 1] = w_theta
    csts[:F, 2] = -w_theta
    csts[F:, 3] = -w_theta
    csts[:F, 4] = wvp
    csts[F:, 5] = wvp
    csts[:F, 6] = wvn
    csts[F:, 7] = wvn

    cw = convh_w @ w_g
    q = (lin_w @ cw) / VT
    r = lin_w.sum(axis=1) / VT
    t = lin_w @ convh_b + lin_b
    return vw, vb, jw, jb, csts, q, r, t


def kernel(**inputs):
    f32 = np.float32
    joint_matrix = inputs.pop("joint_matrix")
    vel_matrix = inputs.pop("vel_matrix")
    vw, vb, jw, jb, csts, q, r, t = _fold(**inputs)
    nc = get_nc(vw, vb, jw, jb)

    import ml_dtypes

    bf16 = ml_dtypes.bfloat16
    per_batch = []
    for b in range(B):
        vel, joint = vel_matrix[b], joint_matrix[b]
        per_batch.append({
            "velZ3": np.ascontiguousarray(vel[:, _Z3_T, _Z3_V].transpose(1, 0, 2), bf16),
            "jntZ3": np.ascontiguousarray(joint[:, _Z3_T, _Z3_V].transpose(1, 0, 2), bf16),
            "velT2": np.ascontiguousarray(vel[:, _T2_T, _T2_V].transpose(1, 0, 2), bf16),
            "jntT2": np.ascontiguousarray(joint[:, _T2_T, _T2_V].transpose(1, 0, 2), bf16),
            "csts": csts.astype(bf16),
        })
    in_maps = [per_batch[k % B] for k in range(8)]

    last_exc = None
    for attempt in range(3):
        try:
            res = run_bass_kernel_spmd(nc, in_maps, core_ids=list(range(8)))
            break
        except Exception as exc:  # transient NRT/device hiccups recover on retry
            last_exc = exc
            if attempt == 2:
                raise
            import time

            time.sleep(10)

    out = np.zeros((B, NCLS), f32)
    for b in range(B):
        outa = res.results[b]["outa"]  # [67, 1] combine partials
        outb = res.results[b]["outb"]  # [128, 2] row sums (bf16)
        Sp = f32(outa.astype(f32).sum())
        sumZ = f32(outb.astype(f32).sum())
        logits = q * (Sp * sumZ) + r * sumZ + t
        e = np.exp(logits - logits.max())
        out[b] = e / e.sum()
    return out.astype(f32)
